# revision 1
# baseline (speedup 1.0000x reference)
"""RNN-T JointNet fused Bass kernel for Trainium2, SPMD over 8 NeuronCores.

Reference computation (all fp32):
    enc = LN(encoder_out @ W_enc + b_enc) * g_enc + be_enc      # [B,T,J]
    dec = LN(decoder_out @ W_dec + b_dec) * g_dec + be_dec      # [B,U,J]
    joint = relu(enc[:,:,None,:] + dec[:,None,:,:])             # [B,T,U,J]
    out = joint @ W_out + b_out                                 # [B,T,U,V]

Shapes: B=4, T=512, U=64, E=D=J=512, V=1024.

Sharding: data-parallel over the flattened (B,T) axis. Core c owns
b = c//2, t in [(c%2)*256, (c%2)*256+256) -> 16384 output rows, which are
contiguous in the flattened [B*T*U, V] output, so the gather is a concat.

Per-core plan (everything stays on-chip between stages):
  - load W_enc/W_dec/W_out K-major, biases broadcast to 128 partitions
  - transpose encoder slice via PE (identity matmul), project with fp32r
    matmuls, layernorm on [t,J] layout (free-dim bn_stats), transpose back
    to J-major encT [128,4,256]; same for decT [128,4,64]
  - for each 512-row supertile: build jointT = relu(encT + decT) with one
    broadcast tensor_tensor add (DVE) + one Relu (ACT); 32 fp32r matmuls
    against resident W_out accumulate [128,512] PSUM tiles; evict
    PSUM->SBUF on alternating DVE/ACT; one 2 MiB DMA to DRAM.
"""

import numpy as np

B, T, U = 4, 512, 64
E = D = J = 512
V = 1024
EPS = 1e-5
P = 128
NCORES = 8
TC = T * B // NCORES            # 256 t-rows per core
ROWS = TC * U                   # 16384 output rows per core
MM_TILES = ROWS // 512          # 32 supertiles of 512 rows (8 t values)

_CACHE = {}


def _build(apply_b_enc, apply_g_enc, apply_be_enc,
           apply_b_dec, apply_g_dec, apply_be_dec, apply_b_out):
    import concourse.bass as bass
    import concourse.mybir as mybir
    import concourse.tile as tile
    from concourse import bacc
    from concourse.masks import make_identity

    f32 = mybir.dt.float32
    f32r = mybir.dt.float32r
    AF = mybir.ActivationFunctionType
    OP = mybir.AluOpType

    nc = bacc.Bacc(target_bir_lowering=False)

    enc_x = nc.dram_tensor("enc_x", [TC, E], f32, kind="ExternalInput")
    dec_x = nc.dram_tensor("dec_x", [U, D], f32, kind="ExternalInput")
    w_enc = nc.dram_tensor("w_enc", [E, J], f32, kind="ExternalInput")
    w_dec = nc.dram_tensor("w_dec", [D, J], f32, kind="ExternalInput")
    w_out = nc.dram_tensor("w_out", [J, V], f32, kind="ExternalInput")
    b_enc = nc.dram_tensor("b_enc", [J], f32, kind="ExternalInput")
    g_enc = nc.dram_tensor("g_enc", [J], f32, kind="ExternalInput")
    be_enc = nc.dram_tensor("be_enc", [J], f32, kind="ExternalInput")
    b_dec = nc.dram_tensor("b_dec", [J], f32, kind="ExternalInput")
    g_dec = nc.dram_tensor("g_dec", [J], f32, kind="ExternalInput")
    be_dec = nc.dram_tensor("be_dec", [J], f32, kind="ExternalInput")
    b_out = nc.dram_tensor("b_out", [V], f32, kind="ExternalInput")
    out = nc.dram_tensor("out", [ROWS, V], f32, kind="ExternalOutput")

    def bcast_row(dram_vec, n):
        # AP that reads a [n] DRAM vector replicated across 128 partitions
        return bass.AP(tensor=dram_vec.tensor, offset=dram_vec.offset,
                       ap=[[0, P], [1, n]])

    from contextlib import ExitStack

    with tile.TileContext(nc) as tc, ExitStack() as ctx:
        const = ctx.enter_context(tc.tile_pool(name="const", bufs=1))
        prep = ctx.enter_context(tc.tile_pool(name="prep", bufs=2))
        jpool = ctx.enter_context(tc.tile_pool(name="jpool", bufs=2))
        jrpool = ctx.enter_context(tc.tile_pool(name="jrpool", bufs=3))
        opool = ctx.enter_context(tc.tile_pool(name="opool", bufs=6))
        mpsum = ctx.enter_context(tc.tile_pool(name="mpsum", bufs=8, space="PSUM"))

        ident = const.tile([P, P], f32)
        make_identity(nc, ident)

        # encoder/decoder input DMAs first so PE transposes can start while
        # the big weight loads stream in
        x_sb = prep.tile([P, TC // P, E], f32, tag="x_sb")
        nc.sync.dma_start(x_sb[:], enc_x[:].rearrange("(o p) e -> p o e", p=P))
        dx_sb = prep.tile([P, D], f32, tag="dx_sb")
        nc.sync.dma_start(dx_sb[:U], dec_x[:])

        wenc_sb = const.tile([P, E // P, J], f32r)
        wdec_sb = const.tile([P, D // P, J], f32r)
        wout_sb = const.tile([P, J // P, V], f32r)
        w_enc_r = w_enc[:].rearrange("(o p) j -> p o j", p=P)
        w_dec_r = w_dec[:].rearrange("(o p) j -> p o j", p=P)
        w_out_r = w_out[:].rearrange("(o p) v -> p o v", p=P)
        for k in range(D // P):
            nc.gpsimd.dma_start(wdec_sb[:, k], w_dec_r[:, k])
            nc.gpsimd.dma_start(wenc_sb[:, k], w_enc_r[:, k])
        for k in range(J // P):
            nc.gpsimd.dma_start(wout_sb[:, k], w_out_r[:, k])

        eps_sb = const.tile([P, 1], f32)
        nc.vector.memset(eps_sb[:], EPS)

        def load_vec(vec, n, enabled):
            if not enabled:
                return None
            t = const.tile([P, n], f32)
            nc.sync.dma_start(t[:], bcast_row(vec, n))
            return t

        b_enc_sb = load_vec(b_enc, J, apply_b_enc)
        g_enc_sb = load_vec(g_enc, J, apply_g_enc)
        be_enc_sb = load_vec(be_enc, J, apply_be_enc)
        b_dec_sb = load_vec(b_dec, J, apply_b_dec)
        g_dec_sb = load_vec(g_dec, J, apply_g_dec)
        be_dec_sb = load_vec(be_dec, J, apply_be_dec)
        b_out_sb = load_vec(b_out, V, apply_b_out)

        encT = const.tile([P, J // P, TC], f32)
        decT = const.tile([P, J // P, U], f32)

        def layer_norm_rows(x_sb, rows, g_sb, be_sb):
            # x_sb: [rows, J] in SBUF; in-place LN over the free dim
            stats = prep.tile([P, 6], f32, tag="ln_stats")
            mv = prep.tile([P, 2], f32, tag="ln_mv")
            nc.vector.bn_stats(out=stats[:rows], in_=x_sb[:rows])
            nc.vector.bn_aggr(out=mv[:rows], in_=stats[:rows])
            rstd = prep.tile([P, 1], f32, tag="ln_rstd")
            nc.scalar.activation(out=rstd[:rows], in_=mv[:rows, 1:2],
                                 func=AF.Sqrt, bias=eps_sb[:rows], scale=1.0)
            nc.vector.reciprocal(out=rstd[:rows], in_=rstd[:rows])
            nc.vector.tensor_scalar(x_sb[:rows], x_sb[:rows],
                                    mv[:rows, 0:1], rstd[:rows],
                                    OP.subtract, OP.mult)
            if g_sb is not None:
                nc.vector.tensor_mul(x_sb[:rows], x_sb[:rows], g_sb[:rows])
            if be_sb is not None:
                nc.vector.tensor_add(x_sb[:rows], x_sb[:rows], be_sb[:rows])

        # ---- decoder side first (small, unblocks the main loop) ----
        dxT = prep.tile([P, D // P, U], f32r, tag="dxT")
        for oe in range(D // P):
            pt = mpsum.tile([P, P], f32, tag="mps", name=f"dpt_{oe}")
            nc.tensor.transpose(pt[:, :U], dx_sb[:U, oe * P:(oe + 1) * P],
                                ident[:U, :U])
            nc.vector.tensor_copy(dxT[:, oe, :], pt[:, :U])
        dps = mpsum.tile([P, J], f32, tag="mps")
        for k in range(D // P):
            nc.tensor.matmul(dps[:U],
                             dxT[:, k, :],
                             wdec_sb[:, k, :],
                             start=(k == 0), stop=(k == D // P - 1))
        decln = prep.tile([P, J], f32, tag="decln")
        if b_dec_sb is not None:
            nc.vector.tensor_add(decln[:U], dps[:U], b_dec_sb[:U])
        else:
            nc.vector.tensor_copy(decln[:U], dps[:U])
        layer_norm_rows(decln, U, g_dec_sb, be_dec_sb)
        for jb in range(J // P):
            pt = mpsum.tile([P, P], f32, tag="mps", name=f"dpt2_{jb}")
            nc.tensor.transpose(pt[:, :U], decln[:U, jb * P:(jb + 1) * P],
                                ident[:U, :U])
            nc.vector.tensor_copy(decT[:, jb, :], pt[:, :U])

        # ---- encoder side, phase A: transposes + projections (PE-dense) ----
        enclns = []
        for tb in range(TC // P):
            xT = prep.tile([P, E // P, P], f32r, tag="xT", name=f"xT_{tb}")
            for oe in range(E // P):
                pt = mpsum.tile([P, P], f32, tag="mps", name=f"ept_{tb}_{oe}")
                nc.tensor.transpose(pt[:], x_sb[:, tb, oe * P:(oe + 1) * P], ident[:])
                nc.vector.tensor_copy(xT[:, oe, :], pt[:])
            eps_mm = mpsum.tile([P, J], f32, tag="mps", name=f"emm_{tb}")
            for k in range(E // P):
                nc.tensor.matmul(eps_mm[:],
                                 xT[:, k, :],
                                 wenc_sb[:, k, :],
                                 start=(k == 0), stop=(k == E // P - 1))
            encln = prep.tile([P, J], f32, tag="encln", name=f"encln_{tb}")
            if b_enc_sb is not None:
                nc.vector.tensor_add(encln[:], eps_mm[:], b_enc_sb[:])
            else:
                nc.vector.tensor_copy(encln[:], eps_mm[:])
            layer_norm_rows(encln, P, g_enc_sb, be_enc_sb)
            enclns.append(encln)
        # phase B: transpose the normalized blocks into encT
        for tb in range(TC // P):
            for jb in range(J // P):
                pt = mpsum.tile([P, P], f32, tag="mps", name=f"ept2_{tb}_{jb}")
                nc.tensor.transpose(pt[:], enclns[tb][:, jb * P:(jb + 1) * P], ident[:])
                nc.vector.tensor_copy(encT[:, jb, tb * P:(tb + 1) * P], pt[:])

        # ---- main loop: 32 supertiles x 512 rows ----
        KJ = J // P          # 4 contraction blocks
        TSUP = 512 // U      # 8 t values per supertile
        out_r = out[:].rearrange("(mm j p) v -> mm p j v", j=4, p=P)
        for mm in range(MM_TILES):
            joint = jpool.tile([P, KJ, 512], f32, tag="joint")
            jr = jrpool.tile([P, KJ, 512], f32r, tag="jr")
            jv = joint.rearrange("p k (t u) -> p k t u", u=U)
            if mm == 0:
                for ch in range(4):
                    tsl = slice(ch * 2, ch * 2 + 2)
                    enc_b = encT[:, :, tsl, None].to_broadcast((P, KJ, 2, U))
                    dec_b = decT[:, :, None, :].to_broadcast((P, KJ, 2, U))
                    nc.vector.tensor_tensor(jv[:, :, tsl], dec_b, enc_b, OP.add)
                    nc.scalar.activation(out=jr[:, :, ch * P:(ch + 1) * P],
                                         in_=joint[:, :, ch * P:(ch + 1) * P],
                                         func=AF.Relu)
            else:
                enc_b = encT[:, :, mm * TSUP:(mm + 1) * TSUP, None] \
                    .to_broadcast((P, KJ, TSUP, U))
                dec_b = decT[:, :, None, :].to_broadcast((P, KJ, TSUP, U))
                nc.gpsimd.tensor_tensor(jv, dec_b, enc_b, OP.add)
                nc.scalar.activation(out=jr[:], in_=joint[:], func=AF.Relu)
            for j in range(4):
                stage = opool.tile([P, V], f32, tag="stage", name=f"st_{mm}_{j}")
                pss = [mpsum.tile([P, 512], f32, tag="mps", name=f"ps_{mm}_{j}_{v}") for v in range(V // 512)]
                for k in range(KJ):
                    for v in range(V // 512):
                        nc.tensor.matmul(
                            pss[v],
                            jr[:, k, j * P:(j + 1) * P],
                            wout_sb[:, k, v * 512:(v + 1) * 512],
                            start=(k == 0), stop=(k == KJ - 1))
                for v in range(V // 512):
                    dst = stage[:, v * 512:(v + 1) * 512]
                    idx = mm * 8 + j * 2 + v
                    if b_out_sb is not None:
                        nc.vector.tensor_add(dst, pss[v][:], b_out_sb[:, v * 512:(v + 1) * 512])
                    elif idx % 8 < 5:
                        nc.vector.tensor_copy(dst, pss[v][:])
                    else:
                        nc.scalar.copy(dst, pss[v][:])
                nc.sync.dma_start(out_r[mm, :, j], stage[:])

    nc.compile()
    return nc


def kernel(**inputs):
    from concourse.bass_utils import run_bass_kernel_spmd

    enc = np.ascontiguousarray(np.asarray(inputs["encoder_out"], dtype=np.float32))
    dec = np.ascontiguousarray(np.asarray(inputs["decoder_out"], dtype=np.float32))
    named = {}
    for k_src, k_dst in [("W_enc", "w_enc"), ("b_enc", "b_enc"), ("g_enc", "g_enc"),
                         ("be_enc", "be_enc"), ("W_dec", "w_dec"), ("b_dec", "b_dec"),
                         ("g_dec", "g_dec"), ("be_dec", "be_dec"),
                         ("W_out", "w_out"), ("b_out", "b_out")]:
        named[k_dst] = np.ascontiguousarray(np.asarray(inputs[k_src], dtype=np.float32))

    flags = (
        bool(np.any(named["b_enc"])), not np.all(named["g_enc"] == 1.0),
        bool(np.any(named["be_enc"])),
        bool(np.any(named["b_dec"])), not np.all(named["g_dec"] == 1.0),
        bool(np.any(named["be_dec"])),
        bool(np.any(named["b_out"])),
    )
    if flags not in _CACHE:
        _CACHE[flags] = _build(*flags)
    nc = _CACHE[flags]

    tpc = T // (NCORES // B)      # t-rows per core
    in_maps = []
    for c in range(NCORES):
        b = c // (NCORES // B)
        t0 = (c % (NCORES // B)) * tpc
        in_maps.append({
            "enc_x": np.ascontiguousarray(enc[b, t0:t0 + tpc]),
            "dec_x": np.ascontiguousarray(dec[b]),
            **named,
        })

    res = run_bass_kernel_spmd(nc, in_maps, core_ids=list(range(NCORES)))
    full = np.concatenate([res.results[c]["out"] for c in range(NCORES)], axis=0)
    return full.reshape(B, T, U, V)



# revision 2
# speedup vs baseline: 1.2817x; 1.2817x over previous
"""RNN-T JointNet fused Bass kernel for Trainium2, SPMD over 8 NeuronCores.

Reference computation (all fp32):
    enc = LN(encoder_out @ W_enc + b_enc) * g_enc + be_enc      # [B,T,J]
    dec = LN(decoder_out @ W_dec + b_dec) * g_dec + be_dec      # [B,U,J]
    joint = relu(enc[:,:,None,:] + dec[:,None,:,:])             # [B,T,U,J]
    out = joint @ W_out + b_out                                 # [B,T,U,V]

Shapes: B=4, T=512, U=64, E=D=J=512, V=1024.

Sharding: data-parallel over the flattened (B,T) axis. Core c owns
b = c//2, t in [(c%2)*256, (c%2)*256+256) -> 16384 output rows, which are
contiguous in the flattened [B*T*U, V] output, so the gather is a concat.

v2 design notes (vs the fp32r baseline):
  - PE column clock is 1 col/cycle @2.4GHz regardless of dtype >= bf16, so
    the main GEMM floor is 1024 matmuls x ~216ns = 221us/core. Everything
    else (joint build, relu, PSUM eviction, output DMA) is sized to hide
    under that.
  - bf16 end-to-end: host pre-casts and pre-transposes inputs/weights to
    bf16 (halves input DMA and removes all phase-A PE transposes), joint
    and jr are bf16 (halves DVE/ACT/Pool traffic), output is written bf16
    (halves output DMA to ~101us) and upcast to fp32 on host.
  - Input DMAs are issued in priority order on one queue (xT, wenc, dxT,
    wdec, wout in 4 chunks) so the projection pipeline starts ~2us in.
  - Joint add is split in halves DVE/Pool, relu halves on ACT, evictions
    alternate DVE/ACT, so each engine stays under the 6.9us/supertile PE
    budget.
"""

import numpy as np

B, T, U = 4, 512, 64
E = D = J = 512
V = 1024
EPS = 1e-5
P = 128
NCORES = 8
TC = T * B // NCORES            # 256 t-rows per core
ROWS = TC * U                   # 16384 output rows per core
MM_TILES = ROWS // 512          # 32 supertiles of 512 rows (8 t values)

_CACHE = {}


def _build(apply_b_enc, apply_g_enc, apply_be_enc,
           apply_b_dec, apply_g_dec, apply_be_dec, apply_b_out):
    import concourse.bass as bass
    import concourse.mybir as mybir
    import concourse.tile as tile
    from concourse import bacc
    from concourse.masks import make_identity

    f32 = mybir.dt.float32
    bf16 = mybir.dt.bfloat16
    AF = mybir.ActivationFunctionType
    OP = mybir.AluOpType

    nc = bacc.Bacc(target_bir_lowering=False)

    # Host supplies pre-transposed, bf16-cast tensors.
    enc_xT = nc.dram_tensor("enc_xT", [E, TC], bf16, kind="ExternalInput")
    dec_xT = nc.dram_tensor("dec_xT", [D, U], bf16, kind="ExternalInput")
    w_enc = nc.dram_tensor("w_enc", [E, J], bf16, kind="ExternalInput")
    w_dec = nc.dram_tensor("w_dec", [D, J], bf16, kind="ExternalInput")
    w_out = nc.dram_tensor("w_out", [J, V], bf16, kind="ExternalInput")
    b_enc = nc.dram_tensor("b_enc", [J], f32, kind="ExternalInput")
    g_enc = nc.dram_tensor("g_enc", [J], f32, kind="ExternalInput")
    be_enc = nc.dram_tensor("be_enc", [J], f32, kind="ExternalInput")
    b_dec = nc.dram_tensor("b_dec", [J], f32, kind="ExternalInput")
    g_dec = nc.dram_tensor("g_dec", [J], f32, kind="ExternalInput")
    be_dec = nc.dram_tensor("be_dec", [J], f32, kind="ExternalInput")
    b_out = nc.dram_tensor("b_out", [V], f32, kind="ExternalInput")
    out = nc.dram_tensor("out", [ROWS, V], bf16, kind="ExternalOutput")

    def bcast_row(dram_vec, n):
        # AP that reads a [n] DRAM vector replicated across 128 partitions
        return bass.AP(tensor=dram_vec.tensor, offset=dram_vec.offset,
                       ap=[[0, P], [1, n]])

    from contextlib import ExitStack

    with tile.TileContext(nc) as tc, ExitStack() as ctx:
        const = ctx.enter_context(tc.tile_pool(name="const", bufs=1))
        prep = ctx.enter_context(tc.tile_pool(name="prep", bufs=2))
        jpool = ctx.enter_context(tc.tile_pool(name="jpool", bufs=2))
        jrpool = ctx.enter_context(tc.tile_pool(name="jrpool", bufs=3))
        opool = ctx.enter_context(tc.tile_pool(name="opool", bufs=6))
        mpsum = ctx.enter_context(tc.tile_pool(name="mpsum", bufs=4, space="PSUM"))

        # ---- input DMAs in priority order on the SP queue ----
        xT_sb = prep.tile([P, E // P, TC], bf16, tag="xT_sb")
        nc.sync.dma_start(xT_sb[:], enc_xT[:].rearrange("(o p) t -> p o t", p=P))
        wenc_sb = const.tile([P, E // P, J], bf16)
        w_enc_r = w_enc[:].rearrange("(o p) j -> p o j", p=P)
        for k in range(E // P):
            nc.sync.dma_start(wenc_sb[:, k], w_enc_r[:, k])
        dxT_sb = prep.tile([P, D // P, U], bf16, tag="dxT_sb")
        nc.sync.dma_start(dxT_sb[:], dec_xT[:].rearrange("(o p) u -> p o u", p=P))
        wdec_sb = const.tile([P, D // P, J], bf16)
        w_dec_r = w_dec[:].rearrange("(o p) j -> p o j", p=P)
        for k in range(D // P):
            nc.sync.dma_start(wdec_sb[:, k], w_dec_r[:, k])
        wout_sb = const.tile([P, J // P, V], bf16)
        w_out_r = w_out[:].rearrange("(o p) v -> p o v", p=P)
        for k in range(J // P):
            nc.sync.dma_start(wout_sb[:, k], w_out_r[:, k])

        ident = const.tile([P, P], f32)
        make_identity(nc, ident)

        eps_sb = const.tile([P, 1], f32)
        nc.vector.memset(eps_sb[:], EPS)

        def load_vec(vec, n, enabled):
            if not enabled:
                return None
            t = const.tile([P, n], f32)
            nc.gpsimd.dma_start(t[:], bcast_row(vec, n))
            return t

        b_enc_sb = load_vec(b_enc, J, apply_b_enc)
        g_enc_sb = load_vec(g_enc, J, apply_g_enc)
        be_enc_sb = load_vec(be_enc, J, apply_be_enc)
        b_dec_sb = load_vec(b_dec, J, apply_b_dec)
        g_dec_sb = load_vec(g_dec, J, apply_g_dec)
        be_dec_sb = load_vec(be_dec, J, apply_be_dec)
        b_out_sb = load_vec(b_out, V, apply_b_out)

        encT = const.tile([P, J // P, TC], bf16)
        decT = const.tile([P, J // P, U], bf16)

        def layer_norm_rows(x_sb, rows, g_sb, be_sb):
            # x_sb: [rows, J] in SBUF; in-place LN over the free dim
            stats = prep.tile([P, 6], f32, tag="ln_stats")
            mv = prep.tile([P, 2], f32, tag="ln_mv")
            nc.vector.bn_stats(out=stats[:rows], in_=x_sb[:rows])
            nc.vector.bn_aggr(out=mv[:rows], in_=stats[:rows])
            rstd = prep.tile([P, 1], f32, tag="ln_rstd")
            nc.scalar.activation(out=rstd[:rows], in_=mv[:rows, 1:2],
                                 func=AF.Sqrt, bias=eps_sb[:rows], scale=1.0)
            nc.vector.reciprocal(out=rstd[:rows], in_=rstd[:rows])
            nc.vector.tensor_scalar(x_sb[:rows], x_sb[:rows],
                                    mv[:rows, 0:1], rstd[:rows],
                                    OP.subtract, OP.mult)
            if g_sb is not None:
                nc.vector.tensor_mul(x_sb[:rows], x_sb[:rows], g_sb[:rows])
            if be_sb is not None:
                nc.vector.tensor_add(x_sb[:rows], x_sb[:rows], be_sb[:rows])

        # ---- encoder projection: [t,J] row-major, LN, transpose to J-major
        for tb in range(TC // P):
            eps_mm = mpsum.tile([P, J], f32, tag="mps", name=f"emm_{tb}")
            for k in range(E // P):
                nc.tensor.matmul(eps_mm[:],
                                 xT_sb[:, k, tb * P:(tb + 1) * P],
                                 wenc_sb[:, k, :],
                                 start=(k == 0), stop=(k == E // P - 1))
            encln = prep.tile([P, J], f32, tag="encln", name=f"encln_{tb}")
            if b_enc_sb is not None:
                nc.vector.tensor_add(encln[:], eps_mm[:], b_enc_sb[:])
            else:
                nc.vector.tensor_copy(encln[:], eps_mm[:])
            layer_norm_rows(encln, P, g_enc_sb, be_enc_sb)
            for jb in range(J // P):
                pt = mpsum.tile([P, P], f32, tag="mps", name=f"ept_{tb}_{jb}")
                nc.tensor.transpose(pt[:], encln[:, jb * P:(jb + 1) * P], ident[:])
                nc.vector.tensor_copy(encT[:, jb, tb * P:(tb + 1) * P], pt[:])

        # ---- decoder projection (small) ----
        dps = mpsum.tile([P, J], f32, tag="mps", name="dmm")
        for k in range(D // P):
            nc.tensor.matmul(dps[:U],
                             dxT_sb[:, k, :],
                             wdec_sb[:, k, :],
                             start=(k == 0), stop=(k == D // P - 1))
        decln = prep.tile([P, J], f32, tag="decln")
        if b_dec_sb is not None:
            nc.vector.tensor_add(decln[:U], dps[:U], b_dec_sb[:U])
        else:
            nc.vector.tensor_copy(decln[:U], dps[:U])
        layer_norm_rows(decln, U, g_dec_sb, be_dec_sb)
        for jb in range(J // P):
            pt = mpsum.tile([P, P], f32, tag="mps", name=f"dpt_{jb}")
            nc.tensor.transpose(pt[:, :U], decln[:U, jb * P:(jb + 1) * P],
                                ident[:U, :U])
            nc.scalar.copy(decT[:, jb, :], pt[:, :U])

        # ---- main loop: 32 supertiles x 512 rows ----
        KJ = J // P          # 4 contraction blocks
        TSUP = 512 // U      # 8 t values per supertile
        out_r = out[:].rearrange("(mm j p) v -> mm p j v", j=4, p=P)
        for mm in range(MM_TILES):
            joint = jpool.tile([P, KJ, 512], bf16, tag="joint")
            jr = jrpool.tile([P, KJ, 512], bf16, tag="jr")
            jv = joint.rearrange("p k (t u) -> p k t u", u=U)
            t0 = mm * TSUP
            # halves: DVE builds h0 (lower latency), Pool builds h1
            for h, eng in ((0, nc.vector), (1, nc.gpsimd)):
                tsl = slice(h * 4, h * 4 + 4)
                enc_b = encT[:, :, t0 + h * 4:t0 + h * 4 + 4, None] \
                    .to_broadcast((P, KJ, 4, U))
                dec_b = decT[:, :, None, :].to_broadcast((P, KJ, 4, U))
                eng.tensor_tensor(jv[:, :, tsl], dec_b, enc_b, OP.add)
                nc.scalar.activation(out=jr[:, :, h * 256:(h + 1) * 256],
                                     in_=joint[:, :, h * 256:(h + 1) * 256],
                                     func=AF.Relu)
            for j in range(4):
                stage = opool.tile([P, V], bf16, tag="stage", name=f"st_{mm}_{j}")
                pss = mpsum.tile([P, V], f32, tag="mps", name=f"ps_{mm}_{j}")
                for k in range(KJ):
                    for v in range(V // 512):
                        nc.tensor.matmul(
                            pss[:, v * 512:(v + 1) * 512],
                            jr[:, k, j * P:(j + 1) * P],
                            wout_sb[:, k, v * 512:(v + 1) * 512],
                            start=(k == 0), stop=(k == KJ - 1))
                if b_out_sb is not None:
                    nc.vector.tensor_add(stage[:, :512], pss[:, :512],
                                         b_out_sb[:, :512])
                    nc.scalar.tensor_add(stage[:, 512:], pss[:, 512:],
                                         b_out_sb[:, 512:])
                elif j % 2 == 0:
                    nc.vector.tensor_copy(stage[:], pss[:])
                else:
                    nc.scalar.copy(stage[:], pss[:])
                nc.sync.dma_start(out_r[mm, :, j], stage[:])

    nc.compile()
    return nc


def kernel(**inputs):
    import ml_dtypes
    from concourse.bass_utils import run_bass_kernel_spmd

    bf = ml_dtypes.bfloat16
    enc = np.asarray(inputs["encoder_out"], dtype=np.float32)
    dec = np.asarray(inputs["decoder_out"], dtype=np.float32)
    named = {}
    for k_src, k_dst in [("b_enc", "b_enc"), ("g_enc", "g_enc"),
                         ("be_enc", "be_enc"), ("b_dec", "b_dec"),
                         ("g_dec", "g_dec"), ("be_dec", "be_dec"),
                         ("b_out", "b_out")]:
        named[k_dst] = np.ascontiguousarray(
            np.asarray(inputs[k_src], dtype=np.float32))
    for k_src, k_dst in [("W_enc", "w_enc"), ("W_dec", "w_dec"),
                         ("W_out", "w_out")]:
        named[k_dst] = np.ascontiguousarray(
            np.asarray(inputs[k_src], dtype=np.float32).astype(bf))

    flags = (
        bool(np.any(named["b_enc"])), not np.all(named["g_enc"] == 1.0),
        bool(np.any(named["be_enc"])),
        bool(np.any(named["b_dec"])), not np.all(named["g_dec"] == 1.0),
        bool(np.any(named["be_dec"])),
        bool(np.any(named["b_out"])),
    )
    if flags not in _CACHE:
        _CACHE[flags] = _build(*flags)
    nc = _CACHE[flags]

    tpc = T // (NCORES // B)      # t-rows per core
    in_maps = []
    for c in range(NCORES):
        b = c // (NCORES // B)
        t0 = (c % (NCORES // B)) * tpc
        in_maps.append({
            "enc_xT": np.ascontiguousarray(enc[b, t0:t0 + tpc].T.astype(bf)),
            "dec_xT": np.ascontiguousarray(dec[b].T.astype(bf)),
            **named,
        })

    res = run_bass_kernel_spmd(nc, in_maps, core_ids=list(range(NCORES)))
    full = np.concatenate(
        [np.asarray(res.results[c]["out"]).astype(np.float32)
         for c in range(NCORES)], axis=0)
    return full.reshape(B, T, U, V)


# revision 4
# speedup vs baseline: 1.3090x; 1.0213x over previous
"""RNN-T JointNet fused Bass kernel for Trainium2, SPMD over 8 NeuronCores.

Reference computation (all fp32):
    enc = LN(encoder_out @ W_enc + b_enc) * g_enc + be_enc      # [B,T,J]
    dec = LN(decoder_out @ W_dec + b_dec) * g_dec + be_dec      # [B,U,J]
    joint = relu(enc[:,:,None,:] + dec[:,None,:,:])             # [B,T,U,J]
    out = joint @ W_out + b_out                                 # [B,T,U,V]

Shapes: B=4, T=512, U=64, E=D=J=512, V=1024.

Sharding: data-parallel over the flattened (B,T) axis. Core c owns
b = c//2, t in [(c%2)*256, (c%2)*256+256) -> 16384 output rows, which are
contiguous in the flattened [B*T*U, V] output, so the gather is a concat.

v2 design notes (vs the fp32r baseline):
  - PE column clock is 1 col/cycle @2.4GHz regardless of dtype >= bf16, so
    the main GEMM floor is 1024 matmuls x ~216ns = 221us/core. Everything
    else (joint build, relu, PSUM eviction, output DMA) is sized to hide
    under that.
  - bf16 end-to-end: host pre-casts and pre-transposes inputs/weights to
    bf16 (halves input DMA and removes all phase-A PE transposes), joint
    and jr are bf16 (halves DVE/ACT/Pool traffic), output is written bf16
    (halves output DMA to ~101us) and upcast to fp32 on host.
  - Input DMAs are issued in priority order on one queue (xT, wenc, dxT,
    wdec, wout in 4 chunks) so the projection pipeline starts ~2us in.
  - Joint add is split in halves DVE/Pool, relu halves on ACT, evictions
    alternate DVE/ACT, so each engine stays under the 6.9us/supertile PE
    budget.
"""

import numpy as np

B, T, U = 4, 512, 64
E = D = J = 512
V = 1024
EPS = 1e-5
P = 128
NCORES = 8
TC = T * B // NCORES            # 256 t-rows per core
ROWS = TC * U                   # 16384 output rows per core
MM_TILES = ROWS // 512          # 32 supertiles of 512 rows (8 t values)

_CACHE = {}


def _build(apply_b_enc, apply_g_enc, apply_be_enc,
           apply_b_dec, apply_g_dec, apply_be_dec, apply_b_out):
    import concourse.bass as bass
    import concourse.mybir as mybir
    import concourse.tile as tile
    from concourse import bacc
    from concourse.masks import make_identity

    f32 = mybir.dt.float32
    bf16 = mybir.dt.bfloat16
    AF = mybir.ActivationFunctionType
    OP = mybir.AluOpType

    nc = bacc.Bacc(target_bir_lowering=False)

    # Host supplies pre-transposed, bf16-cast tensors.
    enc_xT = nc.dram_tensor("enc_xT", [E, TC], bf16, kind="ExternalInput")
    dec_xT = nc.dram_tensor("dec_xT", [D, U], bf16, kind="ExternalInput")
    w_enc = nc.dram_tensor("w_enc", [E, J], bf16, kind="ExternalInput")
    w_dec = nc.dram_tensor("w_dec", [D, J], bf16, kind="ExternalInput")
    w_out = nc.dram_tensor("w_out", [J, V], bf16, kind="ExternalInput")
    b_enc = nc.dram_tensor("b_enc", [J], f32, kind="ExternalInput")
    g_enc = nc.dram_tensor("g_enc", [J], f32, kind="ExternalInput")
    be_enc = nc.dram_tensor("be_enc", [J], f32, kind="ExternalInput")
    b_dec = nc.dram_tensor("b_dec", [J], f32, kind="ExternalInput")
    g_dec = nc.dram_tensor("g_dec", [J], f32, kind="ExternalInput")
    be_dec = nc.dram_tensor("be_dec", [J], f32, kind="ExternalInput")
    b_out = nc.dram_tensor("b_out", [V], f32, kind="ExternalInput")
    out = nc.dram_tensor("out", [ROWS, V], bf16, kind="ExternalOutput")

    def bcast_row(dram_vec, n):
        # AP that reads a [n] DRAM vector replicated across 128 partitions
        return bass.AP(tensor=dram_vec.tensor, offset=dram_vec.offset,
                       ap=[[0, P], [1, n]])

    from contextlib import ExitStack

    with tile.TileContext(nc) as tc, ExitStack() as ctx:
        const = ctx.enter_context(tc.tile_pool(name="const", bufs=1))
        prep = ctx.enter_context(tc.tile_pool(name="prep", bufs=2))
        jpool = ctx.enter_context(tc.tile_pool(name="jpool", bufs=2))
        jrpool = ctx.enter_context(tc.tile_pool(name="jrpool", bufs=3))
        opool = ctx.enter_context(tc.tile_pool(name="opool", bufs=6))
        mpsum = ctx.enter_context(tc.tile_pool(name="mpsum", bufs=4, space="PSUM"))

        # ---- input DMAs in priority order on the SP queue ----
        xT_sb = prep.tile([P, E // P, TC], bf16, tag="xT_sb")
        nc.sync.dma_start(xT_sb[:], enc_xT[:].rearrange("(o p) t -> p o t", p=P))
        wenc_sb = const.tile([P, E // P, J], bf16)
        w_enc_r = w_enc[:].rearrange("(o p) j -> p o j", p=P)
        for k in range(E // P):
            nc.sync.dma_start(wenc_sb[:, k], w_enc_r[:, k])
        dxT_sb = prep.tile([P, D // P, U], bf16, tag="dxT_sb")
        nc.sync.dma_start(dxT_sb[:], dec_xT[:].rearrange("(o p) u -> p o u", p=P))
        wdec_sb = const.tile([P, D // P, J], bf16)
        w_dec_r = w_dec[:].rearrange("(o p) j -> p o j", p=P)
        for k in range(D // P):
            nc.sync.dma_start(wdec_sb[:, k], w_dec_r[:, k])
        wout_sb = const.tile([P, J // P, V], bf16)
        w_out_r = w_out[:].rearrange("(o p) v -> p o v", p=P)
        for k in range(J // P):
            nc.sync.dma_start(wout_sb[:, k], w_out_r[:, k])

        ident = const.tile([P, P], f32)
        make_identity(nc, ident)

        eps_sb = const.tile([P, 1], f32)
        nc.vector.memset(eps_sb[:], EPS)

        def load_vec(vec, n, enabled):
            if not enabled:
                return None
            t = const.tile([P, n], f32)
            nc.gpsimd.dma_start(t[:], bcast_row(vec, n))
            return t

        b_enc_sb = load_vec(b_enc, J, apply_b_enc)
        g_enc_sb = load_vec(g_enc, J, apply_g_enc)
        be_enc_sb = load_vec(be_enc, J, apply_be_enc)
        b_dec_sb = load_vec(b_dec, J, apply_b_dec)
        g_dec_sb = load_vec(g_dec, J, apply_g_dec)
        be_dec_sb = load_vec(be_dec, J, apply_be_dec)
        b_out_sb = load_vec(b_out, V, apply_b_out)

        encT = const.tile([P, J // P, TC], bf16)
        decT = const.tile([P, J // P, U], bf16)

        def layer_norm_rows(x_sb, rows, g_sb, be_sb):
            # x_sb: [rows, J] in SBUF; in-place LN over the free dim
            stats = prep.tile([P, 6], f32, tag="ln_stats")
            mv = prep.tile([P, 2], f32, tag="ln_mv")
            nc.vector.bn_stats(out=stats[:rows], in_=x_sb[:rows])
            nc.vector.bn_aggr(out=mv[:rows], in_=stats[:rows])
            rstd = prep.tile([P, 1], f32, tag="ln_rstd")
            nc.scalar.activation(out=rstd[:rows], in_=mv[:rows, 1:2],
                                 func=AF.Sqrt, bias=eps_sb[:rows], scale=1.0)
            nc.vector.reciprocal(out=rstd[:rows], in_=rstd[:rows])
            nc.vector.tensor_scalar(x_sb[:rows], x_sb[:rows],
                                    mv[:rows, 0:1], rstd[:rows],
                                    OP.subtract, OP.mult)
            if g_sb is not None:
                nc.vector.tensor_mul(x_sb[:rows], x_sb[:rows], g_sb[:rows])
            if be_sb is not None:
                nc.vector.tensor_add(x_sb[:rows], x_sb[:rows], be_sb[:rows])

        # ---- projections first (back-to-back PE work), LNs overlap on DVE
        eps_mms = []
        for tb in range(TC // P):
            eps_mm = mpsum.tile([P, J], f32, tag="mps", name=f"emm_{tb}")
            for k in range(E // P):
                nc.tensor.matmul(eps_mm[:],
                                 xT_sb[:, k, tb * P:(tb + 1) * P],
                                 wenc_sb[:, k, :],
                                 start=(k == 0), stop=(k == E // P - 1))
            eps_mms.append(eps_mm)
        dps = mpsum.tile([P, J], f32, tag="mps", name="dmm")
        for k in range(D // P):
            nc.tensor.matmul(dps[:U],
                             dxT_sb[:, k, :],
                             wdec_sb[:, k, :],
                             start=(k == 0), stop=(k == D // P - 1))

        enclns = []
        for tb in range(TC // P):
            encln = prep.tile([P, J], f32, tag="encln", name=f"encln_{tb}")
            if b_enc_sb is not None:
                nc.vector.tensor_add(encln[:], eps_mms[tb][:], b_enc_sb[:])
            else:
                nc.vector.tensor_copy(encln[:], eps_mms[tb][:])
            layer_norm_rows(encln, P, g_enc_sb, be_enc_sb)
            enclns.append(encln)
        decln = prep.tile([P, J], f32, tag="decln")
        if b_dec_sb is not None:
            nc.vector.tensor_add(decln[:U], dps[:U], b_dec_sb[:U])
        else:
            nc.vector.tensor_copy(decln[:U], dps[:U])
        layer_norm_rows(decln, U, g_dec_sb, be_dec_sb)

        # transposes: tb0 then dec (unblocks the first supertile), tb1 last
        # so it overlaps the first joint build on DVE/ACT
        for jb in range(J // P):
            pt = mpsum.tile([P, P], f32, tag="mps", name=f"ept_0_{jb}")
            nc.tensor.transpose(pt[:], enclns[0][:, jb * P:(jb + 1) * P], ident[:])
            nc.vector.tensor_copy(encT[:, jb, 0:P], pt[:])
        for jb in range(J // P):
            pt = mpsum.tile([P, P], f32, tag="mps", name=f"dpt_{jb}")
            nc.tensor.transpose(pt[:, :U], decln[:U, jb * P:(jb + 1) * P],
                                ident[:U, :U])
            nc.scalar.copy(decT[:, jb, :], pt[:, :U])
        for jb in range(J // P):
            pt = mpsum.tile([P, P], f32, tag="mps", name=f"ept_1_{jb}")
            nc.tensor.transpose(pt[:], enclns[1][:, jb * P:(jb + 1) * P], ident[:])
            nc.vector.tensor_copy(encT[:, jb, P:2 * P], pt[:])

        # ---- main loop: 32 supertiles x 512 rows ----
        KJ = J // P          # 4 contraction blocks
        TSUP = 512 // U      # 8 t values per supertile
        out_r = out[:].rearrange("(mm j p) v -> mm p j v", j=4, p=P)
        for mm in range(MM_TILES):
            joint = jpool.tile([P, KJ, 512], bf16, tag="joint")
            jr = jrpool.tile([P, KJ, 512], bf16, tag="jr")
            jv = joint.rearrange("p k (t u) -> p k t u", u=U)
            t0 = mm * TSUP
            if mm == 0:
                # quarters (one per j-block) to minimize pipeline-fill latency
                for q in range(4):
                    tsl = slice(q * 2, q * 2 + 2)
                    enc_b = encT[:, :, t0 + q * 2:t0 + q * 2 + 2, None] \
                        .to_broadcast((P, KJ, 2, U))
                    dec_b = decT[:, :, None, :].to_broadcast((P, KJ, 2, U))
                    eng = nc.vector if q % 2 == 0 else nc.gpsimd
                    eng.tensor_tensor(jv[:, :, tsl], dec_b, enc_b, OP.add)
                    nc.scalar.activation(out=jr[:, :, q * 128:(q + 1) * 128],
                                         in_=joint[:, :, q * 128:(q + 1) * 128],
                                         func=AF.Relu)
            else:
                # halves: DVE builds h0 (lower latency), Pool builds h1
                for h, eng in ((0, nc.vector), (1, nc.gpsimd)):
                    tsl = slice(h * 4, h * 4 + 4)
                    enc_b = encT[:, :, t0 + h * 4:t0 + h * 4 + 4, None] \
                        .to_broadcast((P, KJ, 4, U))
                    dec_b = decT[:, :, None, :].to_broadcast((P, KJ, 4, U))
                    eng.tensor_tensor(jv[:, :, tsl], dec_b, enc_b, OP.add)
                    nc.scalar.activation(out=jr[:, :, h * 256:(h + 1) * 256],
                                         in_=joint[:, :, h * 256:(h + 1) * 256],
                                         func=AF.Relu)
            for j in range(4):
                stage = opool.tile([P, V], bf16, tag="stage", name=f"st_{mm}_{j}")
                pss = mpsum.tile([P, V], f32, tag="mps", name=f"ps_{mm}_{j}")
                for k in range(KJ):
                    for v in range(V // 512):
                        nc.tensor.matmul(
                            pss[:, v * 512:(v + 1) * 512],
                            jr[:, k, j * P:(j + 1) * P],
                            wout_sb[:, k, v * 512:(v + 1) * 512],
                            start=(k == 0), stop=(k == KJ - 1))
                if b_out_sb is not None:
                    nc.vector.tensor_add(stage[:, :512], pss[:, :512],
                                         b_out_sb[:, :512])
                    nc.scalar.tensor_add(stage[:, 512:], pss[:, 512:],
                                         b_out_sb[:, 512:])
                elif j % 2 == 0:
                    nc.vector.tensor_copy(stage[:], pss[:])
                else:
                    nc.scalar.copy(stage[:], pss[:])
                nc.sync.dma_start(out_r[mm, :, j], stage[:])

    nc.compile()
    return nc


def kernel(**inputs):
    import ml_dtypes
    from concourse.bass_utils import run_bass_kernel_spmd

    bf = ml_dtypes.bfloat16
    enc = np.asarray(inputs["encoder_out"], dtype=np.float32)
    dec = np.asarray(inputs["decoder_out"], dtype=np.float32)
    named = {}
    for k_src, k_dst in [("b_enc", "b_enc"), ("g_enc", "g_enc"),
                         ("be_enc", "be_enc"), ("b_dec", "b_dec"),
                         ("g_dec", "g_dec"), ("be_dec", "be_dec"),
                         ("b_out", "b_out")]:
        named[k_dst] = np.ascontiguousarray(
            np.asarray(inputs[k_src], dtype=np.float32))
    for k_src, k_dst in [("W_enc", "w_enc"), ("W_dec", "w_dec"),
                         ("W_out", "w_out")]:
        named[k_dst] = np.ascontiguousarray(
            np.asarray(inputs[k_src], dtype=np.float32).astype(bf))

    flags = (
        bool(np.any(named["b_enc"])), not np.all(named["g_enc"] == 1.0),
        bool(np.any(named["be_enc"])),
        bool(np.any(named["b_dec"])), not np.all(named["g_dec"] == 1.0),
        bool(np.any(named["be_dec"])),
        bool(np.any(named["b_out"])),
    )
    if flags not in _CACHE:
        _CACHE[flags] = _build(*flags)
    nc = _CACHE[flags]

    tpc = T // (NCORES // B)      # t-rows per core
    in_maps = []
    for c in range(NCORES):
        b = c // (NCORES // B)
        t0 = (c % (NCORES // B)) * tpc
        in_maps.append({
            "enc_xT": np.ascontiguousarray(enc[b, t0:t0 + tpc].T.astype(bf)),
            "dec_xT": np.ascontiguousarray(dec[b].T.astype(bf)),
            **named,
        })

    res = run_bass_kernel_spmd(nc, in_maps, core_ids=list(range(NCORES)))
    full = np.concatenate(
        [np.asarray(res.results[c]["out"]).astype(np.float32)
         for c in range(NCORES)], axis=0)
    return full.reshape(B, T, U, V)


# revision 6
# speedup vs baseline: 1.3243x; 1.0117x over previous
"""RNN-T JointNet fused Bass kernel for Trainium2, SPMD over 8 NeuronCores.

Reference computation (all fp32):
    enc = LN(encoder_out @ W_enc + b_enc) * g_enc + be_enc      # [B,T,J]
    dec = LN(decoder_out @ W_dec + b_dec) * g_dec + be_dec      # [B,U,J]
    joint = relu(enc[:,:,None,:] + dec[:,None,:,:])             # [B,T,U,J]
    out = joint @ W_out + b_out                                 # [B,T,U,V]

Shapes: B=4, T=512, U=64, E=D=J=512, V=1024.

Sharding: data-parallel over the flattened (B,T) axis. Core c owns
b = c//2, t in [(c%2)*256, (c%2)*256+256) -> 16384 output rows, which are
contiguous in the flattened [B*T*U, V] output, so the gather is a concat.

v2 design notes (vs the fp32r baseline):
  - PE column clock is 1 col/cycle @2.4GHz regardless of dtype >= bf16, so
    the main GEMM floor is 1024 matmuls x ~216ns = 221us/core. Everything
    else (joint build, relu, PSUM eviction, output DMA) is sized to hide
    under that.
  - bf16 end-to-end: host pre-casts and pre-transposes inputs/weights to
    bf16 (halves input DMA and removes all phase-A PE transposes), joint
    and jr are bf16 (halves DVE/ACT/Pool traffic), output is written bf16
    (halves output DMA to ~101us) and upcast to fp32 on host.
  - Input DMAs are issued in priority order on one queue (xT, wenc, dxT,
    wdec, wout in 4 chunks) so the projection pipeline starts ~2us in.
  - Joint add is split in halves DVE/Pool, relu halves on ACT, evictions
    alternate DVE/ACT, so each engine stays under the 6.9us/supertile PE
    budget.
"""

import numpy as np

B, T, U = 4, 512, 64
E = D = J = 512
V = 1024
EPS = 1e-5
P = 128
NCORES = 8
TC = T * B // NCORES            # 256 t-rows per core
ROWS = TC * U                   # 16384 output rows per core
MM_TILES = ROWS // 512          # 32 supertiles of 512 rows (8 t values)

_CACHE = {}


def _build(apply_b_enc, apply_g_enc, apply_be_enc,
           apply_b_dec, apply_g_dec, apply_be_dec, apply_b_out):
    import concourse.bass as bass
    import concourse.mybir as mybir
    import concourse.tile as tile
    from concourse import bacc
    from concourse.masks import make_identity

    f32 = mybir.dt.float32
    bf16 = mybir.dt.bfloat16
    AF = mybir.ActivationFunctionType
    OP = mybir.AluOpType

    nc = bacc.Bacc(target_bir_lowering=False)

    # Host supplies pre-transposed, bf16-cast tensors.
    enc_xT = nc.dram_tensor("enc_xT", [E, TC], bf16, kind="ExternalInput")
    dec_xT = nc.dram_tensor("dec_xT", [D, U], bf16, kind="ExternalInput")
    w_enc = nc.dram_tensor("w_enc", [E, J], bf16, kind="ExternalInput")
    w_dec = nc.dram_tensor("w_dec", [D, J], bf16, kind="ExternalInput")
    w_out = nc.dram_tensor("w_out", [J, V], bf16, kind="ExternalInput")
    b_enc = nc.dram_tensor("b_enc", [J], f32, kind="ExternalInput")
    g_enc = nc.dram_tensor("g_enc", [J], f32, kind="ExternalInput")
    be_enc = nc.dram_tensor("be_enc", [J], f32, kind="ExternalInput")
    b_dec = nc.dram_tensor("b_dec", [J], f32, kind="ExternalInput")
    g_dec = nc.dram_tensor("g_dec", [J], f32, kind="ExternalInput")
    be_dec = nc.dram_tensor("be_dec", [J], f32, kind="ExternalInput")
    b_out = nc.dram_tensor("b_out", [V], f32, kind="ExternalInput")
    out = nc.dram_tensor("out", [ROWS, V], bf16, kind="ExternalOutput")

    def bcast_row(dram_vec, n):
        # AP that reads a [n] DRAM vector replicated across 128 partitions
        return bass.AP(tensor=dram_vec.tensor, offset=dram_vec.offset,
                       ap=[[0, P], [1, n]])

    from contextlib import ExitStack

    with tile.TileContext(nc) as tc, ExitStack() as ctx:
        const = ctx.enter_context(tc.tile_pool(name="const", bufs=1))
        prep = ctx.enter_context(tc.tile_pool(name="prep", bufs=2))
        jpool = ctx.enter_context(tc.tile_pool(name="jpool", bufs=2))
        jrpool = ctx.enter_context(tc.tile_pool(name="jrpool", bufs=3))
        opool = ctx.enter_context(tc.tile_pool(name="opool", bufs=6))
        mpsum = ctx.enter_context(tc.tile_pool(name="mpsum", bufs=4, space="PSUM"))

        # ---- input DMAs, priority-ordered on the SP queue (serial per queue,
        # so earlier transfers get full HBM bandwidth) ----
        xT_sb = prep.tile([P, E // P, TC], bf16, tag="xT_sb")
        nc.sync.dma_start(xT_sb[:], enc_xT[:].rearrange("(o p) t -> p o t", p=P))
        wenc_sb = const.tile([P, E // P, J], bf16)
        nc.sync.dma_start(wenc_sb[:], w_enc[:].rearrange("(o p) j -> p o j", p=P))
        dxT_sb = prep.tile([P, D // P, U], bf16, tag="dxT_sb")
        nc.sync.dma_start(dxT_sb[:], dec_xT[:].rearrange("(o p) u -> p o u", p=P))
        wdec_sb = const.tile([P, D // P, J], bf16)
        nc.sync.dma_start(wdec_sb[:], w_dec[:].rearrange("(o p) j -> p o j", p=P))
        wout_sb = const.tile([P, J // P, V], bf16)
        nc.sync.dma_start(wout_sb[:], w_out[:].rearrange("(o p) v -> p o v", p=P))

        ident = const.tile([P, P], bf16)
        make_identity(nc, ident)

        # PE warmup: dummy matmuls on a zeroed tile while input DMAs stream,
        # so the PE p-state is at full clock when the projections start.
        warm_src = const.tile([P, 512], bf16)
        nc.vector.memset(warm_src[:], 0.0)

        def warmup(n, label):
            for i in range(n):
                wps = mpsum.tile([P, 512], f32, tag="mps", name=f"w{label}_{i}")
                nc.tensor.matmul(wps, warm_src[:, :P], warm_src[:],
                                 start=True, stop=True)

        warmup(10, "a")

        eps_sb = const.tile([P, 1], f32)
        nc.vector.memset(eps_sb[:], EPS)

        def load_vec(vec, n, enabled):
            if not enabled:
                return None
            t = const.tile([P, n], f32)
            nc.gpsimd.dma_start(t[:], bcast_row(vec, n))
            return t

        b_enc_sb = load_vec(b_enc, J, apply_b_enc)
        g_enc_sb = load_vec(g_enc, J, apply_g_enc)
        be_enc_sb = load_vec(be_enc, J, apply_be_enc)
        b_dec_sb = load_vec(b_dec, J, apply_b_dec)
        g_dec_sb = load_vec(g_dec, J, apply_g_dec)
        be_dec_sb = load_vec(be_dec, J, apply_be_dec)
        b_out_sb = load_vec(b_out, V, apply_b_out)

        encT = const.tile([P, J // P, TC], bf16)
        decT = const.tile([P, J // P, U], bf16)

        def layer_norm_psum(ps, rows, label, b_sb, g_sb, be_sb):
            """LN over the free dim of psum tile ps [rows, J]; returns a bf16
            SBUF tile with the normalized rows."""
            ln16 = prep.tile([P, J], bf16, tag="ln16", name=f"ln16_{label}")
            if b_sb is not None:
                # legacy path: bias add first, stats on the biased rows
                xf = prep.tile([P, J], f32, tag="lnf", name=f"lnf_{label}")
                nc.vector.tensor_add(xf[:rows], ps[:rows], b_sb[:rows])
                src = xf
            else:
                src = ps
            stats = prep.tile([P, 6], f32, tag="ln_stats", name=f"st_{label}")
            mv = prep.tile([P, 2], f32, tag="ln_mv", name=f"mv_{label}")
            nc.vector.bn_stats(out=stats[:rows], in_=src[:rows])
            nc.vector.bn_aggr(out=mv[:rows], in_=stats[:rows])
            rstd = prep.tile([P, 1], f32, tag="ln_rstd", name=f"rs_{label}")
            nc.scalar.activation(out=rstd[:rows], in_=mv[:rows, 1:2],
                                 func=AF.Sqrt, bias=eps_sb[:rows], scale=1.0)
            nc.vector.reciprocal(out=rstd[:rows], in_=rstd[:rows])
            nc.vector.tensor_scalar(ln16[:rows], src[:rows],
                                    mv[:rows, 0:1], rstd[:rows],
                                    OP.subtract, OP.mult)
            if g_sb is not None:
                nc.vector.tensor_mul(ln16[:rows], ln16[:rows], g_sb[:rows])
            if be_sb is not None:
                nc.vector.tensor_add(ln16[:rows], ln16[:rows], be_sb[:rows])
            return ln16

        # ---- projections (PE) with LN issue interleaved (DVE) ----
        eps_mm0 = mpsum.tile([P, J], f32, tag="mps", name="emm_0")
        for k in range(E // P):
            nc.tensor.matmul(eps_mm0[:], xT_sb[:, k, 0:P], wenc_sb[:, k, :],
                             start=(k == 0), stop=(k == E // P - 1))
        encln0 = layer_norm_psum(eps_mm0, P, "e0", b_enc_sb, g_enc_sb, be_enc_sb)
        dps = mpsum.tile([P, J], f32, tag="mps", name="dmm")
        for k in range(D // P):
            nc.tensor.matmul(dps[:U], dxT_sb[:, k, :], wdec_sb[:, k, :],
                             start=(k == 0), stop=(k == D // P - 1))
        decln = layer_norm_psum(dps, U, "d", b_dec_sb, g_dec_sb, be_dec_sb)
        eps_mm1 = mpsum.tile([P, J], f32, tag="mps", name="emm_1")
        for k in range(E // P):
            nc.tensor.matmul(eps_mm1[:], xT_sb[:, k, P:2 * P], wenc_sb[:, k, :],
                             start=(k == 0), stop=(k == E // P - 1))
        encln1 = layer_norm_psum(eps_mm1, P, "e1", b_enc_sb, g_enc_sb, be_enc_sb)

        # keep the PE busy (p-state) while the first LNs run on DVE
        warmup(8, "b")

        # transposes: tb0 then dec (unblocks the first supertile), tb1 last
        # so it overlaps the first joint build on DVE/ACT
        for jb in range(J // P):
            pt = mpsum.tile([P, P], bf16, tag="mps", name=f"ept_0_{jb}")
            nc.tensor.transpose(pt[:], encln0[:, jb * P:(jb + 1) * P], ident[:])
            nc.scalar.copy(encT[:, jb, 0:P], pt[:])
        for jb in range(J // P):
            pt = mpsum.tile([P, P], bf16, tag="mps", name=f"dpt_{jb}")
            nc.tensor.transpose(pt[:, :U], decln[:U, jb * P:(jb + 1) * P],
                                ident[:U, :U])
            nc.scalar.copy(decT[:, jb, :], pt[:, :U])
        for jb in range(J // P):
            pt = mpsum.tile([P, P], bf16, tag="mps", name=f"ept_1_{jb}")
            nc.tensor.transpose(pt[:], encln1[:, jb * P:(jb + 1) * P], ident[:])
            nc.vector.tensor_copy(encT[:, jb, P:2 * P], pt[:])

        # ---- main loop: 32 supertiles x 512 rows ----
        KJ = J // P          # 4 contraction blocks
        TSUP = 512 // U      # 8 t values per supertile
        out_r = out[:].rearrange("(mm j p) v -> mm p j v", j=4, p=P)
        for mm in range(MM_TILES):
            joint = jpool.tile([P, KJ, 512], bf16, tag="joint")
            jr = jrpool.tile([P, KJ, 512], bf16, tag="jr")
            jv = joint.rearrange("p k (t u) -> p k t u", u=U)
            t0 = mm * TSUP
            if mm == 0:
                # quarters (one per j-block) to minimize pipeline-fill latency
                for q in range(4):
                    tsl = slice(q * 2, q * 2 + 2)
                    enc_b = encT[:, :, t0 + q * 2:t0 + q * 2 + 2, None] \
                        .to_broadcast((P, KJ, 2, U))
                    dec_b = decT[:, :, None, :].to_broadcast((P, KJ, 2, U))
                    eng = nc.vector if q % 2 == 0 else nc.gpsimd
                    eng.tensor_tensor(jv[:, :, tsl], dec_b, enc_b, OP.add)
                    nc.scalar.activation(out=jr[:, :, q * 128:(q + 1) * 128],
                                         in_=joint[:, :, q * 128:(q + 1) * 128],
                                         func=AF.Relu)
            else:
                # halves: DVE builds h0 (lower latency), Pool builds h1
                for h, eng in ((0, nc.vector), (1, nc.gpsimd)):
                    tsl = slice(h * 4, h * 4 + 4)
                    enc_b = encT[:, :, t0 + h * 4:t0 + h * 4 + 4, None] \
                        .to_broadcast((P, KJ, 4, U))
                    dec_b = decT[:, :, None, :].to_broadcast((P, KJ, 4, U))
                    eng.tensor_tensor(jv[:, :, tsl], dec_b, enc_b, OP.add)
                    nc.scalar.activation(out=jr[:, :, h * 256:(h + 1) * 256],
                                         in_=joint[:, :, h * 256:(h + 1) * 256],
                                         func=AF.Relu)
            for j in range(4):
                stage = opool.tile([P, V], bf16, tag="stage", name=f"st_{mm}_{j}")
                pss = mpsum.tile([P, V], f32, tag="mps", name=f"ps_{mm}_{j}")
                for k in range(KJ):
                    for v in range(V // 512):
                        nc.tensor.matmul(
                            pss[:, v * 512:(v + 1) * 512],
                            jr[:, k, j * P:(j + 1) * P],
                            wout_sb[:, k, v * 512:(v + 1) * 512],
                            start=(k == 0), stop=(k == KJ - 1))
                if b_out_sb is not None:
                    nc.vector.tensor_add(stage[:, :512], pss[:, :512],
                                         b_out_sb[:, :512])
                    nc.scalar.tensor_add(stage[:, 512:], pss[:, 512:],
                                         b_out_sb[:, 512:])
                elif j % 2 == 0:
                    nc.vector.tensor_copy(stage[:], pss[:])
                else:
                    nc.scalar.copy(stage[:], pss[:])
                nc.sync.dma_start(out_r[mm, :, j], stage[:])

    nc.compile()
    return nc


def kernel(**inputs):
    import ml_dtypes
    from concourse.bass_utils import run_bass_kernel_spmd

    bf = ml_dtypes.bfloat16
    enc = np.asarray(inputs["encoder_out"], dtype=np.float32)
    dec = np.asarray(inputs["decoder_out"], dtype=np.float32)
    named = {}
    for k_src, k_dst in [("b_enc", "b_enc"), ("g_enc", "g_enc"),
                         ("be_enc", "be_enc"), ("b_dec", "b_dec"),
                         ("g_dec", "g_dec"), ("be_dec", "be_dec"),
                         ("b_out", "b_out")]:
        named[k_dst] = np.ascontiguousarray(
            np.asarray(inputs[k_src], dtype=np.float32))
    for k_src, k_dst in [("W_enc", "w_enc"), ("W_dec", "w_dec"),
                         ("W_out", "w_out")]:
        named[k_dst] = np.ascontiguousarray(
            np.asarray(inputs[k_src], dtype=np.float32).astype(bf))

    flags = (
        bool(np.any(named["b_enc"])), not np.all(named["g_enc"] == 1.0),
        bool(np.any(named["be_enc"])),
        bool(np.any(named["b_dec"])), not np.all(named["g_dec"] == 1.0),
        bool(np.any(named["be_dec"])),
        bool(np.any(named["b_out"])),
    )
    if flags not in _CACHE:
        _CACHE[flags] = _build(*flags)
    nc = _CACHE[flags]

    tpc = T // (NCORES // B)      # t-rows per core
    in_maps = []
    for c in range(NCORES):
        b = c // (NCORES // B)
        t0 = (c % (NCORES // B)) * tpc
        in_maps.append({
            "enc_xT": np.ascontiguousarray(enc[b, t0:t0 + tpc].T.astype(bf)),
            "dec_xT": np.ascontiguousarray(dec[b].T.astype(bf)),
            **named,
        })

    res = run_bass_kernel_spmd(nc, in_maps, core_ids=list(range(NCORES)))
    full = np.concatenate(
        [np.asarray(res.results[c]["out"]).astype(np.float32)
         for c in range(NCORES)], axis=0)
    return full.reshape(B, T, U, V)
